# revision 7
# baseline (speedup 1.0000x reference)
"""GQA attention kernel for Trainium2, 8 NeuronCores (v2).

Sharding: data-parallel over batch (4) x tensor-parallel over head groups (2);
o_proj row-parallel, host sums the 2 partials per batch.

v2 changes vs baseline (cost-model-guided):
  - bf16 everywhere on DVE (RoPE tables + intermediates) -> 2x DVE mode.
  - Diagonal S/O/den chunks narrowed to the causal width (512-128r): saves
    ~7% PE rows and 15% of exp work; single [128,128] triangular 0/1 mask.
  - Softmax denominator reciprocal taken straight from PSUM (bf16);
    broadcast via a bf16 ones-matvec on PE; O evacuated to SBUF early so
    normalization runs in 2x bf16 DVE mode and the o-bank frees fast.
  - Startup DMAs split across both HWDGE queues (x on SP, weights on ACT,
    wq halved between them) in 4-chunk slices; o_proj staged through
    [128,512] PSUM tiles with 3 SBUF out buffers; the last query tile's
    o_proj borrows the idle S-pool banks for deeper rotation.
"""

import json as _json

import numpy as np
import ml_dtypes

import concourse.bass as bass
import concourse.mybir as mybir
import concourse.tile as tile

# --- walrus sync-wait legalizer (same as baseline) -------------------------
_MAX_WAITS = 1
_orig_to_json_bytes = bass.Bass.to_json_bytes


def _split_waits_json(raw: bytes) -> bytes:
    m = _json.loads(raw)
    changed = False
    for fn in m.get("functions", []):
        for bb in fn.get("blocks", []):
            out = []
            for inst in bb.get("instructions", []):
                si = inst.get("sync_info")
                waits = (si or {}).get("on_wait") or []
                if len(waits) > _MAX_WAITS:
                    changed = True
                    for k, w in enumerate(waits[:-_MAX_WAITS]):
                        out.append({
                            "debug": inst.get("debug", 0),
                            "engine": inst["engine"],
                            "ins": [], "outs": [],
                            "name": f"{inst['name']}-sw{k}",
                            "opcode": "EventSemaphore",
                            "sync_info": {"on_update": [], "on_wait": [w]},
                        })
                    si["on_wait"] = waits[-_MAX_WAITS:]
                out.append(inst)
            bb["instructions"] = out
    if not changed:
        return raw
    return _json.dumps(m).encode()


def _patched_to_json_bytes(self):
    return _split_waits_json(_orig_to_json_bytes(self))


bass.Bass.to_json_bytes = _patched_to_json_bytes
# --------------------------------------------------------------------------

B, D = 4, 2048
NH, NKV, HD = 16, 4, 128
NHL, NKVL = 8, 2          # per-core q heads / kv heads
DQ = NHL * HD             # 1024
DKV = NKVL * HD           # 256
KD = D // 128             # 16 contraction chunks
TQ = 512
THETA = 10000.0
SCALE = HD ** -0.5
NCORES = 8

bf16 = mybir.dt.bfloat16
f32 = mybir.dt.float32


def build_nc(T=2048, do_p1=True, do_p2=True, psum_cfg=(4, 1, 1, 1), ev_eng="dve"):
    njq = T // TQ
    nck = T // 128
    ts = bass.ts

    nc = bass.Bass()
    xT = nc.dram_tensor("xT", [D, T], bf16, kind="ExternalInput")
    wq = nc.dram_tensor("wq", [D, DQ], bf16, kind="ExternalInput")
    wk = nc.dram_tensor("wk", [D, DKV], bf16, kind="ExternalInput")
    wv = nc.dram_tensor("wv", [D, DKV], bf16, kind="ExternalInput")
    wo = nc.dram_tensor("wo", [DQ, D], bf16, kind="ExternalInput")
    cosT = nc.dram_tensor("cosT", [HD, T], bf16, kind="ExternalInput")
    sinT = nc.dram_tensor("sinT", [HD, T], bf16, kind="ExternalInput")
    tri = nc.dram_tensor("tri", [128, 128], bf16, kind="ExternalInput")
    out = nc.dram_tensor("out", [T, D], f32, kind="ExternalOutput")

    with tile.TileContext(nc) as tc:
        with tc.tile_pool(name="res", bufs=1) as res:
            QT_sb = res.tile([128, NHL, T], bf16)
            KT_sb = res.tile([128, NKVL, T], bf16)
            V_sb = res.tile([128, nck, DKV], bf16)
            tri_sb = res.tile([128, 128], bf16)
            ones_sb = res.tile([128, 1], bf16)
            onesr_sb = res.tile([1, 128], bf16)
            cos_sb = res.tile([128, T], bf16)
            sin_sb = res.tile([128, T], bf16)

            nc.vector.memset(ones_sb, 1.0)
            nc.vector.memset(onesr_sb, 1.0)
            if not do_p1:
                nc.gpsimd.memset(QT_sb, 0.0)
                nc.gpsimd.memset(KT_sb, 0.0)
                nc.gpsimd.memset(V_sb, 0.0)

            # ---------------- Phase 1: projections + RoPE ----------------
            with tc.tile_pool(name="w1", bufs=1) as w1, \
                 tc.tile_pool(name="p1x", bufs=2) as xpool, \
                 tc.tile_pool(name="p1ps", bufs=2, space="PSUM") as pspool, \
                 tc.tile_pool(name="p1pv", bufs=2, space="PSUM") as pvpool, \
                 tc.tile_pool(name="p1t", bufs=3) as tpool:
                wq_sb = w1.tile([128, KD, DQ], bf16)
                wk_sb = w1.tile([128, KD, DKV], bf16)
                wv_sb = w1.tile([128, KD, DKV], bf16)
                wq_r = wq[:, :].rearrange("(c p) m -> p c m", p=128)
                wk_r = wk[:, :].rearrange("(c p) m -> p c m", p=128)
                nc.scalar.dma_start(out=wk_sb[:, 0:8, :], in_=wk_r[:, 0:8, :])
                nc.scalar.dma_start(out=wk_sb[:, 8:16, :], in_=wk_r[:, 8:16, :])
                nc.scalar.dma_start(out=wv_sb, in_=wv[:, :].rearrange("(c p) m -> p c m", p=128))
                # wq split across BOTH HWDGE queues: first half on ACT
                # behind wk/wv, second half on SP behind the jt=0 x slices
                for c in range(0, KD // 2, 4):
                    nc.scalar.dma_start(out=wq_sb[:, c:c + 4, :],
                                        in_=wq_r[:, c:c + 4, :])

                xT_r = xT[:, :].rearrange("(c p) t -> p c t", p=128)
                for jt in range(njq if do_p1 else 0):
                    xt = xpool.tile([128, KD, TQ], bf16, tag="xt")
                    if jt == 0:  # sliced: K-proj chain starts early
                        for c in range(0, KD, 4):
                            nc.sync.dma_start(out=xt[:, c:c + 4, :],
                                              in_=xT_r[:, c:c + 4, ts(jt, TQ)])
                        for c in range(KD // 2, KD, 4):
                            nc.sync.dma_start(out=wq_sb[:, c:c + 4, :],
                                              in_=wq_r[:, c:c + 4, :])
                        # constants: nothing needs them until RoPE / the
                        # first diagonal mask
                        nc.scalar.dma_start(out=tri_sb, in_=tri[:, :])
                        nc.scalar.dma_start(out=cos_sb, in_=cosT[:, :])
                        nc.scalar.dma_start(out=sin_sb, in_=sinT[:, :])
                    else:
                        nc.sync.dma_start(out=xt, in_=xT_r[:, :, ts(jt, TQ)])
                    # K heads first (small weights arrive first), then V,
                    # then Q (wq streams in behind)
                    for h in [NHL, NHL + 1, -1] + list(range(NHL)):
                        if h == -1:  # V blocks here, between K and Q
                            for s in range(4):
                                pv = pvpool.tile([128, DKV], f32, tag="pv")
                                for c in range(KD):
                                    nc.tensor.matmul(
                                        pv,
                                        lhsT=xt[:, c, s * 128:(s + 1) * 128],
                                        rhs=wv_sb[:, c, :],
                                        start=(c == 0), stop=(c == KD - 1))
                                nc.scalar.copy(V_sb[:, 4 * jt + s, :], pv)
                            continue
                        if h < NHL:
                            w_sb, col = wq_sb, h * 128
                            dst = QT_sb[:, h, ts(jt, TQ)]
                        else:
                            g = h - NHL
                            w_sb, col = wk_sb, g * 128
                            dst = KT_sb[:, g, ts(jt, TQ)]
                        ps = pspool.tile([128, TQ], f32, tag="ps")
                        for c in range(KD):
                            nc.tensor.matmul(ps, lhsT=w_sb[:, c, col:col + 128],
                                             rhs=xt[:, c, :],
                                             start=(c == 0), stop=(c == KD - 1))
                        # RoPE (bf16): dst = qf*cos + shift64(qf)*sin_signed
                        qf = tpool.tile([128, TQ], bf16, tag="qf")
                        nc.scalar.copy(qf, ps)
                        qs = tpool.tile([128, TQ], bf16, tag="qs")
                        nc.sync.dma_start(out=qs[0:64, :], in_=qf[64:128, :])
                        nc.sync.dma_start(out=qs[64:128, :], in_=qf[0:64, :])
                        t1 = tpool.tile([128, TQ], bf16, tag="t1")
                        nc.vector.tensor_mul(t1, qf, cos_sb[:, ts(jt, TQ)])
                        nc.vector.tensor_mul(qs, qs, sin_sb[:, ts(jt, TQ)])
                        nc.vector.tensor_add(dst, t1, qs)

            # ---------------- Phase 2: attention + o_proj ----------------
            with tc.tile_pool(name="w2", bufs=1) as w2, \
                 tc.tile_pool(name="p2s", bufs=psum_cfg[0], space="PSUM") as spool, \
                 tc.tile_pool(name="p2o", bufs=psum_cfg[1], space="PSUM") as opool, \
                 tc.tile_pool(name="p2d", bufs=psum_cfg[2], space="PSUM") as dpool, \
                 tc.tile_pool(name="p2bc", bufs=1, space="PSUM") as bcpool, \
                 tc.tile_pool(name="p2op", bufs=psum_cfg[3], space="PSUM") as oppool, \
                 tc.tile_pool(name="p2p", bufs=4) as ppool, \
                 tc.tile_pool(name="p2t", bufs=2) as t2pool, \
                 tc.tile_pool(name="p2ot", bufs=2) as otpool, \
                 tc.tile_pool(name="p2out", bufs=3) as outpool:
                wo_sb = w2.tile([128, NHL, D], bf16)
                nc.sync.dma_start(out=wo_sb, in_=wo[:, :].rearrange("(c p) n -> p c n", p=128))

                for jq in range(njq if do_p2 else 0):
                    OT = otpool.tile([128, NHL, TQ], bf16, tag="OT")
                    for h in range(NHL):
                        g = h // 4
                        nch = 4 * jq + 4
                        o_ps = opool.tile([128, TQ], f32, tag="o")
                        d_ps = dpool.tile([1, TQ], f32, tag="d")
                        for c in range(nch):
                            r = c - 4 * jq
                            if r < 0:       # full chunk
                                w, off = TQ, 0
                            else:           # diagonal chunk: causal-narrowed
                                w, off = TQ - 128 * r, 128 * r
                            s_ps = spool.tile([128, w], f32, tag="s")
                            nc.tensor.matmul(
                                s_ps,
                                lhsT=KT_sb[:, g, c * 128:(c + 1) * 128],
                                rhs=QT_sb[:, h, jq * TQ + off: (jq + 1) * TQ],
                                start=True, stop=True)
                            p_sb = ppool.tile([128, w], bf16, tag="p")
                            nc.scalar.activation(p_sb, s_ps,
                                                 mybir.ActivationFunctionType.Exp,
                                                 scale=SCALE)
                            if r >= 0:  # mask the leading 128 cols (diag block)
                                nc.vector.tensor_mul(p_sb[:, 0:128],
                                                     p_sb[:, 0:128], tri_sb)
                            nc.tensor.matmul(o_ps[:, off:TQ],
                                             lhsT=V_sb[:, c, g * 128:(g + 1) * 128],
                                             rhs=p_sb,
                                             start=(c == 0), stop=(c == nch - 1))
                            nc.tensor.matmul(d_ps[:, off:TQ], lhsT=ones_sb,
                                             rhs=p_sb,
                                             start=(c == 0), stop=(c == nch - 1))
                        # evacuate unnormalized O early (frees the o bank),
                        # normalize in SBUF where DVE runs in 2x bf16 mode
                        o_sb = t2pool.tile([128, TQ], bf16, tag="onorm")
                        if ev_eng == "act":
                            nc.scalar.copy(o_sb, o_ps)
                        else:
                            nc.vector.tensor_copy(o_sb, o_ps)
                        rden = t2pool.tile([1, TQ], bf16, tag="rden")
                        with nc.allow_low_precision(reason="softmax denom reciprocal in bf16"):
                            nc.vector.reciprocal(rden, d_ps)
                        bc_ps = bcpool.tile([128, TQ], f32, tag="bc")
                        nc.tensor.matmul(bc_ps, lhsT=onesr_sb, rhs=rden,
                                         start=True, stop=True)
                        bc_sb = t2pool.tile([128, TQ], bf16, tag="bc")
                        nc.scalar.copy(bc_sb, bc_ps)
                        nc.vector.tensor_mul(OT[:, h, :], o_sb, bc_sb)
                    # o_proj for this query tile; the last tile borrows the
                    # S pool's banks (attention is done) for deeper rotation
                    for s in range(4):
                        for nt in range(4):
                            if jq == njq - 1 and (s * 4 + nt) % 5 < 3:
                                op_ps = spool.tile([128, 512], f32, tag="s",
                                                   name=f"opl{s}{nt}")
                            else:
                                op_ps = oppool.tile([128, 512], f32, tag="op")
                            for hc in range(NHL):
                                nc.tensor.matmul(
                                    op_ps,
                                    lhsT=OT[:, hc, s * 128:(s + 1) * 128],
                                    rhs=wo_sb[:, hc, nt * 512:(nt + 1) * 512],
                                    start=(hc == 0), stop=(hc == NHL - 1))
                            osb = outpool.tile([128, 512], f32, tag="osb")
                            nc.vector.tensor_copy(osb, op_ps)
                            row = jq * TQ + s * 128
                            nc.sync.dma_start(
                                out=out[row:row + 128,
                                        nt * 512:(nt + 1) * 512],
                                in_=osb)
    return nc


def rope_tables(T=2048):
    inv = 1.0 / (THETA ** (np.arange(0, HD, 2, dtype=np.float32) / HD))
    t = np.arange(T, dtype=np.float32)
    freqs = np.outer(t, inv)
    emb = np.concatenate([freqs, freqs], -1)      # [T, 128]
    cos = np.ascontiguousarray(np.cos(emb).T).astype(ml_dtypes.bfloat16)
    sin = np.sin(emb).T.astype(np.float32)
    sin_signed = sin.copy()
    sin_signed[:64] *= -1.0                        # rotate_half sign fold
    return cos, np.ascontiguousarray(sin_signed.astype(ml_dtypes.bfloat16))


def tri_mask():
    k = np.arange(128)[:, None]
    q = np.arange(128)[None, :]
    return np.ascontiguousarray((k <= q).astype(ml_dtypes.bfloat16))


def build_in_maps(x, wq, wk, wv, wo, T=2048):
    bf = ml_dtypes.bfloat16
    cos, sin_s = rope_tables(T)
    trim = tri_mask()
    wq16 = np.asarray(wq).astype(bf)
    wk16 = np.asarray(wk).astype(bf)
    wv16 = np.asarray(wv).astype(bf)
    wo16 = np.asarray(wo).astype(bf)
    x = np.asarray(x)
    xTb = [np.ascontiguousarray(x[b].T).astype(bf) for b in range(B)]
    wq_h = [np.ascontiguousarray(wq16[:, hg * DQ:(hg + 1) * DQ]) for hg in range(2)]
    wk_h = [np.ascontiguousarray(wk16[:, hg * DKV:(hg + 1) * DKV]) for hg in range(2)]
    wv_h = [np.ascontiguousarray(wv16[:, hg * DKV:(hg + 1) * DKV]) for hg in range(2)]
    wo_h = [np.ascontiguousarray(wo16[hg * DQ:(hg + 1) * DQ, :]) for hg in range(2)]
    in_maps = []
    for core in range(NCORES):
        b, hg = core // 2, core % 2
        in_maps.append({
            "xT": xTb[b],
            "wq": wq_h[hg], "wk": wk_h[hg], "wv": wv_h[hg], "wo": wo_h[hg],
            "cosT": cos, "sinT": sin_s, "tri": trim,
        })
    return in_maps


_NC_CACHE = {}


def get_nc(T=2048):
    if T not in _NC_CACHE:
        _NC_CACHE[T] = build_nc(T)
    return _NC_CACHE[T]


def run(inputs, trace=False, **kw):
    """Returns (full_output [B,T,D] f32, BassKernelResults)."""
    from concourse import bass_utils
    x = np.asarray(inputs["x"], dtype=np.float32)
    T = x.shape[1]
    nc = get_nc(T)
    in_maps = build_in_maps(x, inputs["wq"], inputs["wk"], inputs["wv"],
                            inputs["wo"], T)
    res = bass_utils.run_bass_kernel_spmd(nc, in_maps,
                                          core_ids=list(range(NCORES)),
                                          trace=trace, **kw)
    outs = [np.asarray(r["out"]) for r in res.results]
    full = np.empty((B, T, D), dtype=np.float32)
    for b in range(B):
        full[b] = outs[2 * b] + outs[2 * b + 1]
    return full, res


def kernel(x, mask, wq, wk, wv, wo):
    full, _ = run({"x": x, "mask": mask, "wq": wq, "wk": wk, "wv": wv, "wo": wo})
    return full


# revision 8
# speedup vs baseline: 1.0087x; 1.0087x over previous
"""GQA attention kernel for Trainium2, 8 NeuronCores (v2).

Sharding: data-parallel over batch (4) x tensor-parallel over head groups (2);
o_proj row-parallel, host sums the 2 partials per batch.

v2 changes vs baseline (cost-model-guided):
  - bf16 everywhere on DVE (RoPE tables + intermediates) -> 2x DVE mode.
  - Diagonal S/O/den chunks narrowed to the causal width (512-128r): saves
    ~7% PE rows and 15% of exp work; single [128,128] triangular 0/1 mask.
  - Softmax denominator reciprocal taken straight from PSUM (bf16);
    broadcast via a bf16 ones-matvec on PE; O evacuated to SBUF early so
    normalization runs in 2x bf16 DVE mode and the o-bank frees fast.
  - Startup DMAs split across both HWDGE queues (x on SP, weights on ACT,
    wq halved between them) in 4-chunk slices; o_proj staged through
    [128,512] PSUM tiles with 3 SBUF out buffers; the last query tile's
    o_proj borrows the idle S-pool banks for deeper rotation.
"""

import json as _json

import numpy as np
import ml_dtypes

import concourse.bass as bass
import concourse.mybir as mybir
import concourse.tile as tile

# --- walrus sync-wait legalizer (same as baseline) -------------------------
_MAX_WAITS = 1
_orig_to_json_bytes = bass.Bass.to_json_bytes


def _split_waits_json(raw: bytes) -> bytes:
    m = _json.loads(raw)
    changed = False
    for fn in m.get("functions", []):
        for bb in fn.get("blocks", []):
            out = []
            for inst in bb.get("instructions", []):
                si = inst.get("sync_info")
                waits = (si or {}).get("on_wait") or []
                if len(waits) > _MAX_WAITS:
                    changed = True
                    for k, w in enumerate(waits[:-_MAX_WAITS]):
                        out.append({
                            "debug": inst.get("debug", 0),
                            "engine": inst["engine"],
                            "ins": [], "outs": [],
                            "name": f"{inst['name']}-sw{k}",
                            "opcode": "EventSemaphore",
                            "sync_info": {"on_update": [], "on_wait": [w]},
                        })
                    si["on_wait"] = waits[-_MAX_WAITS:]
                out.append(inst)
            bb["instructions"] = out
    if not changed:
        return raw
    return _json.dumps(m).encode()


def _patched_to_json_bytes(self):
    return _split_waits_json(_orig_to_json_bytes(self))


bass.Bass.to_json_bytes = _patched_to_json_bytes
# --------------------------------------------------------------------------

B, D = 4, 2048
NH, NKV, HD = 16, 4, 128
NHL, NKVL = 8, 2          # per-core q heads / kv heads
DQ = NHL * HD             # 1024
DKV = NKVL * HD           # 256
KD = D // 128             # 16 contraction chunks
TQ = 512
THETA = 10000.0
SCALE = HD ** -0.5
NCORES = 8

bf16 = mybir.dt.bfloat16
f32 = mybir.dt.float32


def build_nc(T=2048, do_p1=True, do_p2=True, psum_cfg=(4, 1, 1, 1), ev_eng="act"):
    njq = T // TQ
    nck = T // 128
    ts = bass.ts

    nc = bass.Bass()
    xT = nc.dram_tensor("xT", [D, T], bf16, kind="ExternalInput")
    wq = nc.dram_tensor("wq", [D, DQ], bf16, kind="ExternalInput")
    wk = nc.dram_tensor("wk", [D, DKV], bf16, kind="ExternalInput")
    wv = nc.dram_tensor("wv", [D, DKV], bf16, kind="ExternalInput")
    wo = nc.dram_tensor("wo", [DQ, D], bf16, kind="ExternalInput")
    cosT = nc.dram_tensor("cosT", [HD, T], bf16, kind="ExternalInput")
    sinT = nc.dram_tensor("sinT", [HD, T], bf16, kind="ExternalInput")
    tri = nc.dram_tensor("tri", [128, 128], bf16, kind="ExternalInput")
    out = nc.dram_tensor("out", [T, D], f32, kind="ExternalOutput")

    with tile.TileContext(nc) as tc:
        with tc.tile_pool(name="res", bufs=1) as res:
            QT_sb = res.tile([128, NHL, T], bf16)
            KT_sb = res.tile([128, NKVL, T], bf16)
            V_sb = res.tile([128, nck, DKV], bf16)
            tri_sb = res.tile([128, 128], bf16)
            ones_sb = res.tile([128, 1], bf16)
            onesr_sb = res.tile([1, 128], bf16)
            cos_sb = res.tile([128, T], bf16)
            sin_sb = res.tile([128, T], bf16)

            nc.vector.memset(ones_sb, 1.0)
            nc.vector.memset(onesr_sb, 1.0)
            if not do_p1:
                nc.gpsimd.memset(QT_sb, 0.0)
                nc.gpsimd.memset(KT_sb, 0.0)
                nc.gpsimd.memset(V_sb, 0.0)

            # ---------------- Phase 1: projections + RoPE ----------------
            with tc.tile_pool(name="w1", bufs=1) as w1, \
                 tc.tile_pool(name="p1x", bufs=2) as xpool, \
                 tc.tile_pool(name="p1ps", bufs=2, space="PSUM") as pspool, \
                 tc.tile_pool(name="p1pv", bufs=2, space="PSUM") as pvpool, \
                 tc.tile_pool(name="p1t", bufs=3) as tpool:
                wq_sb = w1.tile([128, KD, DQ], bf16)
                wk_sb = w1.tile([128, KD, DKV], bf16)
                wv_sb = w1.tile([128, KD, DKV], bf16)
                wq_r = wq[:, :].rearrange("(c p) m -> p c m", p=128)
                wk_r = wk[:, :].rearrange("(c p) m -> p c m", p=128)
                nc.scalar.dma_start(out=wk_sb[:, 0:8, :], in_=wk_r[:, 0:8, :])
                nc.scalar.dma_start(out=wk_sb[:, 8:16, :], in_=wk_r[:, 8:16, :])
                nc.scalar.dma_start(out=wv_sb, in_=wv[:, :].rearrange("(c p) m -> p c m", p=128))
                # wq split across BOTH HWDGE queues: first half on ACT
                # behind wk/wv, second half on SP behind the jt=0 x slices
                for c in range(0, KD // 2, 4):
                    nc.scalar.dma_start(out=wq_sb[:, c:c + 4, :],
                                        in_=wq_r[:, c:c + 4, :])

                xT_r = xT[:, :].rearrange("(c p) t -> p c t", p=128)
                for jt in range(njq if do_p1 else 0):
                    xt = xpool.tile([128, KD, TQ], bf16, tag="xt")
                    if jt == 0:  # sliced: K-proj chain starts early
                        for c in range(0, KD, 4):
                            nc.sync.dma_start(out=xt[:, c:c + 4, :],
                                              in_=xT_r[:, c:c + 4, ts(jt, TQ)])
                        for c in range(KD // 2, KD, 4):
                            nc.sync.dma_start(out=wq_sb[:, c:c + 4, :],
                                              in_=wq_r[:, c:c + 4, :])
                        # constants: nothing needs them until RoPE / the
                        # first diagonal mask
                        nc.scalar.dma_start(out=tri_sb, in_=tri[:, :])
                        nc.scalar.dma_start(out=cos_sb, in_=cosT[:, :])
                        nc.scalar.dma_start(out=sin_sb, in_=sinT[:, :])
                    else:
                        nc.sync.dma_start(out=xt, in_=xT_r[:, :, ts(jt, TQ)])
                    # K heads first (small weights arrive first), then V,
                    # then Q (wq streams in behind)
                    for h in [NHL, NHL + 1, -1] + list(range(NHL)):
                        if h == -1:  # V blocks here, between K and Q
                            for s in range(4):
                                pv = pvpool.tile([128, DKV], f32, tag="pv")
                                for c in range(KD):
                                    nc.tensor.matmul(
                                        pv,
                                        lhsT=xt[:, c, s * 128:(s + 1) * 128],
                                        rhs=wv_sb[:, c, :],
                                        start=(c == 0), stop=(c == KD - 1))
                                nc.scalar.copy(V_sb[:, 4 * jt + s, :], pv)
                            continue
                        if h < NHL:
                            w_sb, col = wq_sb, h * 128
                            dst = QT_sb[:, h, ts(jt, TQ)]
                        else:
                            g = h - NHL
                            w_sb, col = wk_sb, g * 128
                            dst = KT_sb[:, g, ts(jt, TQ)]
                        ps = pspool.tile([128, TQ], f32, tag="ps")
                        for c in range(KD):
                            nc.tensor.matmul(ps, lhsT=w_sb[:, c, col:col + 128],
                                             rhs=xt[:, c, :],
                                             start=(c == 0), stop=(c == KD - 1))
                        # RoPE (bf16): dst = qf*cos + shift64(qf)*sin_signed
                        qf = tpool.tile([128, TQ], bf16, tag="qf")
                        nc.scalar.copy(qf, ps)
                        qs = tpool.tile([128, TQ], bf16, tag="qs")
                        nc.sync.dma_start(out=qs[0:64, :], in_=qf[64:128, :])
                        nc.sync.dma_start(out=qs[64:128, :], in_=qf[0:64, :])
                        t1 = tpool.tile([128, TQ], bf16, tag="t1")
                        nc.vector.tensor_mul(t1, qf, cos_sb[:, ts(jt, TQ)])
                        nc.vector.tensor_mul(qs, qs, sin_sb[:, ts(jt, TQ)])
                        nc.vector.tensor_add(dst, t1, qs)

            # ---------------- Phase 2: attention + o_proj ----------------
            with tc.tile_pool(name="w2", bufs=1) as w2, \
                 tc.tile_pool(name="p2s", bufs=psum_cfg[0], space="PSUM") as spool, \
                 tc.tile_pool(name="p2o", bufs=psum_cfg[1], space="PSUM") as opool, \
                 tc.tile_pool(name="p2d", bufs=psum_cfg[2], space="PSUM") as dpool, \
                 tc.tile_pool(name="p2bc", bufs=1, space="PSUM") as bcpool, \
                 tc.tile_pool(name="p2op", bufs=psum_cfg[3], space="PSUM") as oppool, \
                 tc.tile_pool(name="p2p", bufs=4) as ppool, \
                 tc.tile_pool(name="p2t", bufs=2) as t2pool, \
                 tc.tile_pool(name="p2ot", bufs=2) as otpool, \
                 tc.tile_pool(name="p2out", bufs=3) as outpool:
                wo_sb = w2.tile([128, NHL, D], bf16)
                nc.sync.dma_start(out=wo_sb, in_=wo[:, :].rearrange("(c p) n -> p c n", p=128))

                for jq in range(njq if do_p2 else 0):
                    OT = otpool.tile([128, NHL, TQ], bf16, tag="OT")
                    for h in range(NHL):
                        g = h // 4
                        nch = 4 * jq + 4
                        o_ps = opool.tile([128, TQ], f32, tag="o")
                        d_ps = dpool.tile([1, TQ], f32, tag="d")
                        for c in range(nch):
                            r = c - 4 * jq
                            if r < 0:       # full chunk
                                w, off = TQ, 0
                            else:           # diagonal chunk: causal-narrowed
                                w, off = TQ - 128 * r, 128 * r
                            s_ps = spool.tile([128, w], f32, tag="s")
                            nc.tensor.matmul(
                                s_ps,
                                lhsT=KT_sb[:, g, c * 128:(c + 1) * 128],
                                rhs=QT_sb[:, h, jq * TQ + off: (jq + 1) * TQ],
                                start=True, stop=True)
                            p_sb = ppool.tile([128, w], bf16, tag="p")
                            nc.scalar.activation(p_sb, s_ps,
                                                 mybir.ActivationFunctionType.Exp,
                                                 scale=SCALE)
                            if r >= 0:  # mask the leading 128 cols (diag block)
                                nc.vector.tensor_mul(p_sb[:, 0:128],
                                                     p_sb[:, 0:128], tri_sb)
                            nc.tensor.matmul(o_ps[:, off:TQ],
                                             lhsT=V_sb[:, c, g * 128:(g + 1) * 128],
                                             rhs=p_sb,
                                             start=(c == 0), stop=(c == nch - 1))
                            nc.tensor.matmul(d_ps[:, off:TQ], lhsT=ones_sb,
                                             rhs=p_sb,
                                             start=(c == 0), stop=(c == nch - 1))
                        # evacuate unnormalized O early (frees the o bank),
                        # normalize in SBUF where DVE runs in 2x bf16 mode
                        o_sb = t2pool.tile([128, TQ], bf16, tag="onorm")
                        if ev_eng == "act":
                            nc.scalar.copy(o_sb, o_ps)
                        else:
                            nc.vector.tensor_copy(o_sb, o_ps)
                        rden = t2pool.tile([1, TQ], bf16, tag="rden")
                        with nc.allow_low_precision(reason="softmax denom reciprocal in bf16"):
                            nc.vector.reciprocal(rden, d_ps)
                        bc_ps = bcpool.tile([128, TQ], f32, tag="bc")
                        nc.tensor.matmul(bc_ps, lhsT=onesr_sb, rhs=rden,
                                         start=True, stop=True)
                        bc_sb = t2pool.tile([128, TQ], bf16, tag="bc")
                        nc.vector.tensor_copy(bc_sb, bc_ps)
                        nc.vector.tensor_mul(OT[:, h, :], o_sb, bc_sb)
                    # o_proj for this query tile; the last tile borrows the
                    # S pool's banks (attention is done) for deeper rotation
                    for s in range(4):
                        for nt in range(4):
                            if jq == njq - 1 and (s * 4 + nt) % 5 < 4:
                                op_ps = spool.tile([128, 512], f32, tag="s",
                                                   name=f"opl{s}{nt}")
                            else:
                                op_ps = oppool.tile([128, 512], f32, tag="op")
                            for hc in range(NHL):
                                nc.tensor.matmul(
                                    op_ps,
                                    lhsT=OT[:, hc, s * 128:(s + 1) * 128],
                                    rhs=wo_sb[:, hc, nt * 512:(nt + 1) * 512],
                                    start=(hc == 0), stop=(hc == NHL - 1))
                            osb = outpool.tile([128, 512], f32, tag="osb")
                            nc.vector.tensor_copy(osb, op_ps)
                            row = jq * TQ + s * 128
                            nc.sync.dma_start(
                                out=out[row:row + 128,
                                        nt * 512:(nt + 1) * 512],
                                in_=osb)
    return nc


def rope_tables(T=2048):
    inv = 1.0 / (THETA ** (np.arange(0, HD, 2, dtype=np.float32) / HD))
    t = np.arange(T, dtype=np.float32)
    freqs = np.outer(t, inv)
    emb = np.concatenate([freqs, freqs], -1)      # [T, 128]
    cos = np.ascontiguousarray(np.cos(emb).T).astype(ml_dtypes.bfloat16)
    sin = np.sin(emb).T.astype(np.float32)
    sin_signed = sin.copy()
    sin_signed[:64] *= -1.0                        # rotate_half sign fold
    return cos, np.ascontiguousarray(sin_signed.astype(ml_dtypes.bfloat16))


def tri_mask():
    k = np.arange(128)[:, None]
    q = np.arange(128)[None, :]
    return np.ascontiguousarray((k <= q).astype(ml_dtypes.bfloat16))


def build_in_maps(x, wq, wk, wv, wo, T=2048):
    bf = ml_dtypes.bfloat16
    cos, sin_s = rope_tables(T)
    trim = tri_mask()
    wq16 = np.asarray(wq).astype(bf)
    wk16 = np.asarray(wk).astype(bf)
    wv16 = np.asarray(wv).astype(bf)
    wo16 = np.asarray(wo).astype(bf)
    x = np.asarray(x)
    xTb = [np.ascontiguousarray(x[b].T).astype(bf) for b in range(B)]
    wq_h = [np.ascontiguousarray(wq16[:, hg * DQ:(hg + 1) * DQ]) for hg in range(2)]
    wk_h = [np.ascontiguousarray(wk16[:, hg * DKV:(hg + 1) * DKV]) for hg in range(2)]
    wv_h = [np.ascontiguousarray(wv16[:, hg * DKV:(hg + 1) * DKV]) for hg in range(2)]
    wo_h = [np.ascontiguousarray(wo16[hg * DQ:(hg + 1) * DQ, :]) for hg in range(2)]
    in_maps = []
    for core in range(NCORES):
        b, hg = core // 2, core % 2
        in_maps.append({
            "xT": xTb[b],
            "wq": wq_h[hg], "wk": wk_h[hg], "wv": wv_h[hg], "wo": wo_h[hg],
            "cosT": cos, "sinT": sin_s, "tri": trim,
        })
    return in_maps


_NC_CACHE = {}


def get_nc(T=2048):
    if T not in _NC_CACHE:
        _NC_CACHE[T] = build_nc(T)
    return _NC_CACHE[T]


def run(inputs, trace=False, **kw):
    """Returns (full_output [B,T,D] f32, BassKernelResults)."""
    from concourse import bass_utils
    x = np.asarray(inputs["x"], dtype=np.float32)
    T = x.shape[1]
    nc = get_nc(T)
    in_maps = build_in_maps(x, inputs["wq"], inputs["wk"], inputs["wv"],
                            inputs["wo"], T)
    res = bass_utils.run_bass_kernel_spmd(nc, in_maps,
                                          core_ids=list(range(NCORES)),
                                          trace=trace, **kw)
    outs = [np.asarray(r["out"]) for r in res.results]
    full = np.empty((B, T, D), dtype=np.float32)
    for b in range(B):
        full[b] = outs[2 * b] + outs[2 * b + 1]
    return full, res


def kernel(x, mask, wq, wk, wv, wo):
    full, _ = run({"x": x, "mask": mask, "wq": wq, "wk": wk, "wv": wv, "wo": wo})
    return full


# revision 9
# speedup vs baseline: 1.0100x; 1.0012x over previous
"""GQA attention kernel for Trainium2, 8 NeuronCores (v2).

Sharding: data-parallel over batch (4) x tensor-parallel over head groups (2);
o_proj row-parallel, host sums the 2 partials per batch.

v2 changes vs baseline (cost-model-guided):
  - bf16 everywhere on DVE (RoPE tables + intermediates) -> 2x DVE mode.
  - Diagonal S/O/den chunks narrowed to the causal width (512-128r): saves
    ~7% PE rows and 15% of exp work; single [128,128] triangular 0/1 mask.
  - Softmax denominator reciprocal taken straight from PSUM (bf16);
    broadcast via a bf16 ones-matvec on PE; O evacuated to SBUF early so
    normalization runs in 2x bf16 DVE mode and the o-bank frees fast.
  - Startup DMAs split across both HWDGE queues (x on SP, weights on ACT,
    wq halved between them) in 4-chunk slices; o_proj staged through
    [128,512] PSUM tiles with 3 SBUF out buffers; the last query tile's
    o_proj borrows the idle S-pool banks for deeper rotation.
"""

import json as _json

import numpy as np
import ml_dtypes

import concourse.bass as bass
import concourse.mybir as mybir
import concourse.tile as tile

# --- walrus sync-wait legalizer (same as baseline) -------------------------
_MAX_WAITS = 1
_orig_to_json_bytes = bass.Bass.to_json_bytes


def _split_waits_json(raw: bytes) -> bytes:
    m = _json.loads(raw)
    changed = False
    for fn in m.get("functions", []):
        for bb in fn.get("blocks", []):
            out = []
            for inst in bb.get("instructions", []):
                si = inst.get("sync_info")
                waits = (si or {}).get("on_wait") or []
                if len(waits) > _MAX_WAITS:
                    changed = True
                    for k, w in enumerate(waits[:-_MAX_WAITS]):
                        out.append({
                            "debug": inst.get("debug", 0),
                            "engine": inst["engine"],
                            "ins": [], "outs": [],
                            "name": f"{inst['name']}-sw{k}",
                            "opcode": "EventSemaphore",
                            "sync_info": {"on_update": [], "on_wait": [w]},
                        })
                    si["on_wait"] = waits[-_MAX_WAITS:]
                out.append(inst)
            bb["instructions"] = out
    if not changed:
        return raw
    return _json.dumps(m).encode()


def _patched_to_json_bytes(self):
    return _split_waits_json(_orig_to_json_bytes(self))


bass.Bass.to_json_bytes = _patched_to_json_bytes
# --------------------------------------------------------------------------

B, D = 4, 2048
NH, NKV, HD = 16, 4, 128
NHL, NKVL = 8, 2          # per-core q heads / kv heads
DQ = NHL * HD             # 1024
DKV = NKVL * HD           # 256
KD = D // 128             # 16 contraction chunks
TQ = 512
THETA = 10000.0
SCALE = HD ** -0.5
NCORES = 8

bf16 = mybir.dt.bfloat16
f32 = mybir.dt.float32


def build_nc(T=2048, do_p1=True, do_p2=True, psum_cfg=(4, 1, 1, 1), ev_eng="act"):
    njq = T // TQ
    nck = T // 128
    ts = bass.ts

    nc = bass.Bass()
    xT = nc.dram_tensor("xT", [D, T], bf16, kind="ExternalInput")
    wq = nc.dram_tensor("wq", [D, DQ], bf16, kind="ExternalInput")
    wk = nc.dram_tensor("wk", [D, DKV], bf16, kind="ExternalInput")
    wv = nc.dram_tensor("wv", [D, DKV], bf16, kind="ExternalInput")
    wo = nc.dram_tensor("wo", [DQ, D], bf16, kind="ExternalInput")
    cosT = nc.dram_tensor("cosT", [HD, T], bf16, kind="ExternalInput")
    sinT = nc.dram_tensor("sinT", [HD, T], bf16, kind="ExternalInput")
    tri = nc.dram_tensor("tri", [128, 128], bf16, kind="ExternalInput")
    out = nc.dram_tensor("out", [T, D], f32, kind="ExternalOutput")

    with tile.TileContext(nc) as tc:
        with tc.tile_pool(name="res", bufs=1) as res:
            QT_sb = res.tile([128, NHL, T], bf16)
            KT_sb = res.tile([128, NKVL, T], bf16)
            V_sb = res.tile([128, nck, DKV], bf16)
            tri_sb = res.tile([128, 128], bf16)
            ones_sb = res.tile([128, 1], bf16)
            onesr_sb = res.tile([1, 128], bf16)
            cos_sb = res.tile([128, T], bf16)
            sin_sb = res.tile([128, T], bf16)

            nc.vector.memset(ones_sb, 1.0)
            nc.vector.memset(onesr_sb, 1.0)
            if not do_p1:
                nc.gpsimd.memset(QT_sb, 0.0)
                nc.gpsimd.memset(KT_sb, 0.0)
                nc.gpsimd.memset(V_sb, 0.0)

            # ---------------- Phase 1: projections + RoPE ----------------
            with tc.tile_pool(name="w1", bufs=1) as w1, \
                 tc.tile_pool(name="p1x", bufs=2) as xpool, \
                 tc.tile_pool(name="p1ps", bufs=2, space="PSUM") as pspool, \
                 tc.tile_pool(name="p1pv", bufs=2, space="PSUM") as pvpool, \
                 tc.tile_pool(name="p1t", bufs=3) as tpool:
                wq_sb = w1.tile([128, KD, DQ], bf16)
                wk_sb = w1.tile([128, KD, DKV], bf16)
                wv_sb = w1.tile([128, KD, DKV], bf16)
                wq_r = wq[:, :].rearrange("(c p) m -> p c m", p=128)
                wk_r = wk[:, :].rearrange("(c p) m -> p c m", p=128)
                nc.scalar.dma_start(out=wk_sb[:, 0:8, :], in_=wk_r[:, 0:8, :])
                nc.scalar.dma_start(out=wk_sb[:, 8:16, :], in_=wk_r[:, 8:16, :])
                nc.scalar.dma_start(out=wv_sb, in_=wv[:, :].rearrange("(c p) m -> p c m", p=128))
                # wq split across BOTH HWDGE queues: first half on ACT
                # behind wk/wv, second half on SP behind the jt=0 x slices
                for c in range(0, KD // 2, 4):
                    nc.scalar.dma_start(out=wq_sb[:, c:c + 4, :],
                                        in_=wq_r[:, c:c + 4, :])

                xT_r = xT[:, :].rearrange("(c p) t -> p c t", p=128)
                for jt in range(njq if do_p1 else 0):
                    xt = xpool.tile([128, KD, TQ], bf16, tag="xt")
                    if jt == 0:  # sliced: K-proj chain starts early
                        for c in range(0, KD, 4):
                            nc.sync.dma_start(out=xt[:, c:c + 4, :],
                                              in_=xT_r[:, c:c + 4, ts(jt, TQ)])
                        for c in range(KD // 2, KD, 4):
                            nc.sync.dma_start(out=wq_sb[:, c:c + 4, :],
                                              in_=wq_r[:, c:c + 4, :])
                        # constants: nothing needs them until RoPE / the
                        # first diagonal mask
                        nc.scalar.dma_start(out=tri_sb, in_=tri[:, :])
                        nc.scalar.dma_start(out=cos_sb, in_=cosT[:, :])
                        nc.scalar.dma_start(out=sin_sb, in_=sinT[:, :])
                    else:
                        nc.sync.dma_start(out=xt, in_=xT_r[:, :, ts(jt, TQ)])
                    # K heads first (small weights arrive first), then V,
                    # then Q (wq streams in behind)
                    for h in [NHL, NHL + 1, -1] + list(range(NHL)):
                        if h == -1:  # V blocks here, between K and Q
                            for s in range(4):
                                pv = pvpool.tile([128, DKV], f32, tag="pv")
                                for c in range(KD):
                                    nc.tensor.matmul(
                                        pv,
                                        lhsT=xt[:, c, s * 128:(s + 1) * 128],
                                        rhs=wv_sb[:, c, :],
                                        start=(c == 0), stop=(c == KD - 1))
                                nc.scalar.copy(V_sb[:, 4 * jt + s, :], pv)
                            continue
                        if h < NHL:
                            w_sb, col = wq_sb, h * 128
                            dst = QT_sb[:, h, ts(jt, TQ)]
                        else:
                            g = h - NHL
                            w_sb, col = wk_sb, g * 128
                            dst = KT_sb[:, g, ts(jt, TQ)]
                        ps = pspool.tile([128, TQ], f32, tag="ps")
                        for c in range(KD):
                            nc.tensor.matmul(ps, lhsT=w_sb[:, c, col:col + 128],
                                             rhs=xt[:, c, :],
                                             start=(c == 0), stop=(c == KD - 1))
                        # RoPE (bf16): dst = qf*cos + shift64(qf)*sin_signed
                        qf = tpool.tile([128, TQ], bf16, tag="qf")
                        nc.scalar.copy(qf, ps)
                        qs = tpool.tile([128, TQ], bf16, tag="qs")
                        nc.sync.dma_start(out=qs[0:64, :], in_=qf[64:128, :])
                        nc.sync.dma_start(out=qs[64:128, :], in_=qf[0:64, :])
                        t1 = tpool.tile([128, TQ], bf16, tag="t1")
                        nc.vector.tensor_mul(t1, qf, cos_sb[:, ts(jt, TQ)])
                        nc.vector.tensor_mul(qs, qs, sin_sb[:, ts(jt, TQ)])
                        nc.vector.tensor_add(dst, t1, qs)

            # ---------------- Phase 2: attention + o_proj ----------------
            with tc.tile_pool(name="w2", bufs=1) as w2, \
                 tc.tile_pool(name="p2s", bufs=psum_cfg[0], space="PSUM") as spool, \
                 tc.tile_pool(name="p2o", bufs=psum_cfg[1], space="PSUM") as opool, \
                 tc.tile_pool(name="p2d", bufs=psum_cfg[2], space="PSUM") as dpool, \
                 tc.tile_pool(name="p2bc", bufs=1, space="PSUM") as bcpool, \
                 tc.tile_pool(name="p2op", bufs=psum_cfg[3], space="PSUM") as oppool, \
                 tc.tile_pool(name="p2p", bufs=6) as ppool, \
                 tc.tile_pool(name="p2t", bufs=2) as t2pool, \
                 tc.tile_pool(name="p2ot", bufs=2) as otpool, \
                 tc.tile_pool(name="p2out", bufs=3) as outpool:
                wo_sb = w2.tile([128, NHL, D], bf16)
                nc.sync.dma_start(out=wo_sb, in_=wo[:, :].rearrange("(c p) n -> p c n", p=128))

                for jq in range(njq if do_p2 else 0):
                    OT = otpool.tile([128, NHL, TQ], bf16, tag="OT")
                    for h in range(NHL):
                        g = h // 4
                        nch = 4 * jq + 4
                        o_ps = opool.tile([128, TQ], f32, tag="o")
                        d_ps = dpool.tile([1, TQ], f32, tag="d")
                        pend = None  # O/den shifted one chunk later in
                        # emission order: the previous head's tail gets a
                        # full chunk of slack before o/d head-block the PE
                        for c in range(nch):
                            r = c - 4 * jq
                            if r < 0:       # full chunk
                                w, off = TQ, 0
                            else:           # diagonal chunk: causal-narrowed
                                w, off = TQ - 128 * r, 128 * r
                            s_ps = spool.tile([128, w], f32, tag="s")
                            nc.tensor.matmul(
                                s_ps,
                                lhsT=KT_sb[:, g, c * 128:(c + 1) * 128],
                                rhs=QT_sb[:, h, jq * TQ + off: (jq + 1) * TQ],
                                start=True, stop=True)
                            p_sb = ppool.tile([128, w], bf16, tag="p")
                            nc.scalar.activation(p_sb, s_ps,
                                                 mybir.ActivationFunctionType.Exp,
                                                 scale=SCALE)
                            if r >= 0:  # mask the leading 128 cols (diag block)
                                nc.vector.tensor_mul(p_sb[:, 0:128],
                                                     p_sb[:, 0:128], tri_sb)
                            if pend is not None:
                                pend()
                            def pend(c=c, p_sb=p_sb, off=off):
                                nc.tensor.matmul(o_ps[:, off:TQ],
                                                 lhsT=V_sb[:, c, g * 128:(g + 1) * 128],
                                                 rhs=p_sb,
                                                 start=(c == 0), stop=(c == nch - 1))
                                nc.tensor.matmul(d_ps[:, off:TQ], lhsT=ones_sb,
                                                 rhs=p_sb,
                                                 start=(c == 0), stop=(c == nch - 1))
                        pend()
                        # evacuate unnormalized O early (frees the o bank),
                        # normalize in SBUF where DVE runs in 2x bf16 mode
                        o_sb = t2pool.tile([128, TQ], bf16, tag="onorm")
                        if ev_eng == "act":
                            nc.scalar.copy(o_sb, o_ps)
                        else:
                            nc.vector.tensor_copy(o_sb, o_ps)
                        rden = t2pool.tile([1, TQ], bf16, tag="rden")
                        with nc.allow_low_precision(reason="softmax denom reciprocal in bf16"):
                            nc.vector.reciprocal(rden, d_ps)
                        bc_ps = bcpool.tile([128, TQ], f32, tag="bc")
                        nc.tensor.matmul(bc_ps, lhsT=onesr_sb, rhs=rden,
                                         start=True, stop=True)
                        bc_sb = t2pool.tile([128, TQ], bf16, tag="bc")
                        nc.vector.tensor_copy(bc_sb, bc_ps)
                        nc.vector.tensor_mul(OT[:, h, :], o_sb, bc_sb)
                    # o_proj for this query tile; the last tile borrows the
                    # S pool's banks (attention is done) for deeper rotation
                    for s in range(4):
                        for nt in range(4):
                            if jq == njq - 1 and (s * 4 + nt) % 5 < 4:
                                op_ps = spool.tile([128, 512], f32, tag="s",
                                                   name=f"opl{s}{nt}")
                            else:
                                op_ps = oppool.tile([128, 512], f32, tag="op")
                            for hc in range(NHL):
                                nc.tensor.matmul(
                                    op_ps,
                                    lhsT=OT[:, hc, s * 128:(s + 1) * 128],
                                    rhs=wo_sb[:, hc, nt * 512:(nt + 1) * 512],
                                    start=(hc == 0), stop=(hc == NHL - 1))
                            osb = outpool.tile([128, 512], f32, tag="osb")
                            nc.vector.tensor_copy(osb, op_ps)
                            row = jq * TQ + s * 128
                            nc.sync.dma_start(
                                out=out[row:row + 128,
                                        nt * 512:(nt + 1) * 512],
                                in_=osb)
    return nc


def rope_tables(T=2048):
    inv = 1.0 / (THETA ** (np.arange(0, HD, 2, dtype=np.float32) / HD))
    t = np.arange(T, dtype=np.float32)
    freqs = np.outer(t, inv)
    emb = np.concatenate([freqs, freqs], -1)      # [T, 128]
    cos = np.ascontiguousarray(np.cos(emb).T).astype(ml_dtypes.bfloat16)
    sin = np.sin(emb).T.astype(np.float32)
    sin_signed = sin.copy()
    sin_signed[:64] *= -1.0                        # rotate_half sign fold
    return cos, np.ascontiguousarray(sin_signed.astype(ml_dtypes.bfloat16))


def tri_mask():
    k = np.arange(128)[:, None]
    q = np.arange(128)[None, :]
    return np.ascontiguousarray((k <= q).astype(ml_dtypes.bfloat16))


def build_in_maps(x, wq, wk, wv, wo, T=2048):
    bf = ml_dtypes.bfloat16
    cos, sin_s = rope_tables(T)
    trim = tri_mask()
    wq16 = np.asarray(wq).astype(bf)
    wk16 = np.asarray(wk).astype(bf)
    wv16 = np.asarray(wv).astype(bf)
    wo16 = np.asarray(wo).astype(bf)
    x = np.asarray(x)
    xTb = [np.ascontiguousarray(x[b].T).astype(bf) for b in range(B)]
    wq_h = [np.ascontiguousarray(wq16[:, hg * DQ:(hg + 1) * DQ]) for hg in range(2)]
    wk_h = [np.ascontiguousarray(wk16[:, hg * DKV:(hg + 1) * DKV]) for hg in range(2)]
    wv_h = [np.ascontiguousarray(wv16[:, hg * DKV:(hg + 1) * DKV]) for hg in range(2)]
    wo_h = [np.ascontiguousarray(wo16[hg * DQ:(hg + 1) * DQ, :]) for hg in range(2)]
    in_maps = []
    for core in range(NCORES):
        b, hg = core // 2, core % 2
        in_maps.append({
            "xT": xTb[b],
            "wq": wq_h[hg], "wk": wk_h[hg], "wv": wv_h[hg], "wo": wo_h[hg],
            "cosT": cos, "sinT": sin_s, "tri": trim,
        })
    return in_maps


_NC_CACHE = {}


def get_nc(T=2048):
    if T not in _NC_CACHE:
        _NC_CACHE[T] = build_nc(T)
    return _NC_CACHE[T]


def run(inputs, trace=False, **kw):
    """Returns (full_output [B,T,D] f32, BassKernelResults)."""
    from concourse import bass_utils
    x = np.asarray(inputs["x"], dtype=np.float32)
    T = x.shape[1]
    nc = get_nc(T)
    in_maps = build_in_maps(x, inputs["wq"], inputs["wk"], inputs["wv"],
                            inputs["wo"], T)
    res = bass_utils.run_bass_kernel_spmd(nc, in_maps,
                                          core_ids=list(range(NCORES)),
                                          trace=trace, **kw)
    outs = [np.asarray(r["out"]) for r in res.results]
    full = np.empty((B, T, D), dtype=np.float32)
    for b in range(B):
        full[b] = outs[2 * b] + outs[2 * b + 1]
    return full, res


def kernel(x, mask, wq, wk, wv, wo):
    full, _ = run({"x": x, "mask": mask, "wq": wq, "wk": wk, "wv": wv, "wo": wo})
    return full
